# revision 21
# baseline (speedup 1.0000x reference)
"""Trainium2 Bass kernel for nn_Decoder_without_dropout (2-layer GRU + attention).

Strategy (8 NeuronCores):
- The GRU recurrence (T=64 steps) is inherently sequential and its per-step
  matmuls stream the full weight matrices through the PE regardless of batch,
  so it is REPLICATED on all 8 cores (per-step cross-core collectives measured
  ~13us each - more than the redundant compute they would save).
- Everything parallel is sharded: layer-0 input projections gi0 by time
  (8 steps/core, gathered via 4 chunked AllGathers that overlap compute),
  attention + output projection by batch (16 elems/core).
- The attention/output block never feeds the recurrence, so it is deferred
  and batched after the 64 steps.
- dtypes: recurrence matmuls fp16 (all values are small, so fp16's 11-bit
  mantissa beats bf16 8x in noise; sim-validated out err ~8e-3), state
  carried fp32, attention scores/softmax/context in plain fp32 (softmax
  amplifies score noise), q1/output projection in fp32r (HW-measured
  1.4e-4 rel err).
"""

import sys

import numpy as np

sys.path.insert(0, "/opt/trn_rl_repo")

import concourse.bass as bass  # noqa: E402
import concourse.mybir as mybir  # noqa: E402
import concourse.tile as tile  # noqa: E402
from concourse import bacc  # noqa: E402
from concourse.alu_op_type import AluOpType  # noqa: E402
from concourse.bass_utils import run_bass_kernel_spmd  # noqa: E402
from concourse.masks import make_identity  # noqa: E402

T, B, HID, EMB = 64, 128, 1024, 1024
G = 3 * HID  # 3072 gate width
S = 128  # attention source length
NCORES = 8
TSH = T // NCORES  # 8 timesteps per core (gi0 shard)
BSH = B // NCORES  # 16 batch elems per core (attention shard)
NK = HID // 128  # 8 contraction chunks
NAG = 4  # gi0 AllGather chunks

F32 = mybir.dt.float32
F32R = mybir.dt.float32r
FP16 = mybir.dt.float16
AF = mybir.ActivationFunctionType
ADD = AluOpType.add
SUB = AluOpType.subtract
MUL = AluOpType.mult

_BUILD_CACHE = {}


def build(t_steps=T):
    nc = bacc.Bacc(
        "TRN2", target_bir_lowering=False, debug=False, num_devices=NCORES
    )

    # ---- inputs (per-core) ----
    embT_in = nc.dram_tensor("embT", [EMB, TSH * B], FP16, kind="ExternalInput")
    wih0T_in = nc.dram_tensor("wih0T", [HID, G], FP16, kind="ExternalInput")
    whh0T_in = nc.dram_tensor("whh0T", [HID, G], FP16, kind="ExternalInput")
    wih1T_in = nc.dram_tensor("wih1T", [HID, G], FP16, kind="ExternalInput")
    whh1T_in = nc.dram_tensor("whh1T", [HID, G], FP16, kind="ExternalInput")
    bias0_in = nc.dram_tensor("bias0", [128, G], FP16, kind="ExternalInput")
    bias1rz_in = nc.dram_tensor("bias1rz", [128, 2048], FP16, kind="ExternalInput")
    bih1n_in = nc.dram_tensor("bih1n", [128, HID], FP16, kind="ExternalInput")
    bhh0n_in = nc.dram_tensor("bhh0n", [128, HID], FP16, kind="ExternalInput")
    bhh1n_in = nc.dram_tensor("bhh1n", [128, HID], FP16, kind="ExternalInput")
    hT_in = nc.dram_tensor("hT", [2, HID, B], FP16, kind="ExternalInput")
    h_in = nc.dram_tensor("h0", [2, B, HID], F32, kind="ExternalInput")
    L1T_in = nc.dram_tensor("L1T", [HID, HID], F32R, kind="ExternalInput")
    L2T_in = nc.dram_tensor("L2T", [2 * HID, HID], F32R, kind="ExternalInput")
    # HcT[b][h][s] = H[s, bsl[b], h]   (scores rhs)
    HcT_in = nc.dram_tensor("HcT", [BSH, HID, S], F32, kind="ExternalInput")
    # Hc[b][s][h] = H[s, bsl[b], h]    (context lhsT)
    Hc_in = nc.dram_tensor("Hc", [BSH, S, HID], F32, kind="ExternalInput")
    # per-core row indices into o_nat [T*B, HID]: column m holds the 128 rows
    # (8 timesteps x 16 local batch) of phase-D row-tile m
    oidx_in = nc.dram_tensor("oidx", [128, 8], mybir.dt.int32, kind="ExternalInput")

    # ---- outputs ----
    out_dram = nc.dram_tensor("out", [T, BSH, HID], F32, kind="ExternalOutput")
    hfin_dram = nc.dram_tensor("hfin", [2, B, HID], F32, kind="ExternalOutput")

    with tile.TileContext(nc, num_cores=NCORES) as tc:
        with (
            tc.tile_pool(name="dram", bufs=1, space="DRAM") as dram,
            tc.tile_pool(name="consts", bufs=1) as consts,
        ):
            gi0_self = dram.tile([TSH * B, G], FP16)
            # chunk j gathers rows [256j, 256j+256) of every core's gi0_self;
            # step t reads chunk (t%8)//2 at row (t//8)*256 + (t%8%2)*128
            gi0_ch = [
                dram.tile(
                    [NCORES * 256, G], FP16, addr_space="Shared", name=f"gi0ch{j}"
                )
                for j in range(NAG)
            ]
            o_nat = dram.tile([T * B, HID], F32)  # o_t = h1(t), natural layout

            ident = consts.tile([128, 128], F32)
            make_identity(nc, ident)

            # ========= PHASE A: gi0 for my 8 timesteps + chunked AG ====
            with (
                tc.tile_pool(name="pa_w", bufs=1) as pa_w,
                tc.tile_pool(name="pa_sb", bufs=2) as pa_sb,
                tc.tile_pool(name="pa_ps", bufs=1, space="PSUM") as pa_ps,
            ):
                wih0 = []
                for k in range(NK):
                    wt = pa_w.tile([128, G], FP16, tag=f"wih0_{k}")
                    nc.sync.dma_start(wt[:], wih0T_in[k * 128 : (k + 1) * 128, :])
                    wih0.append(wt)
                bias0 = pa_w.tile([128, G], FP16, tag="bias0")
                nc.sync.dma_start(bias0[:], bias0_in[:])

                for rt in range(TSH):  # row tiles of 128 (one t, all b)
                    ets = []
                    for k in range(NK):
                        et = pa_sb.tile([128, 128], FP16, tag=f"et{k}")
                        nc.sync.dma_start(
                            et[:],
                            embT_in[k * 128 : (k + 1) * 128, rt * 128 : (rt + 1) * 128],
                        )
                        ets.append(et)
                    # k-outer: one stationary load serves all 6 output chunks
                    ps6 = [
                        pa_ps.tile([128, 512], F32, tag=f"pa{i}", name=f"ps6_{i}")
                        for i in range(6)
                    ]
                    for k in range(NK):
                        for i in range(6):
                            nc.tensor.matmul(
                                ps6[i][:],
                                ets[k][:],
                                wih0[k][:, i * 512 : (i + 1) * 512],
                                start=(k == 0),
                                stop=(k == NK - 1),
                            )
                    for i in range(6):
                        ob = pa_sb.tile([128, 512], FP16, tag="ob")
                        nc.vector.tensor_tensor(
                            ob[:], ps6[i][:], bias0[:, i * 512 : (i + 1) * 512], op=ADD
                        )
                        nc.sync.dma_start(
                            gi0_self[
                                rt * 128 : (rt + 1) * 128, i * 512 : (i + 1) * 512
                            ],
                            ob[:],
                        )
                    if rt % 2 == 1:
                        j = rt // 2
                        nc.gpsimd.collective_compute(
                            "AllGather",
                            AluOpType.bypass,
                            replica_groups=[list(range(NCORES))],
                            ins=[gi0_self[256 * j : 256 * (j + 1), :]],
                            outs=[gi0_ch[j].opt()],
                        )

            # ================= PHASE C: recurrence =====================
            with (
                tc.tile_pool(name="pc_w", bufs=1) as pc_w,
                tc.tile_pool(name="pc_pf", bufs=1) as pc_pf,
                tc.tile_pool(name="pc_g", bufs=1) as pc_g,
                tc.tile_pool(name="pc_tmp", bufs=1) as pc_tmp,
                tc.tile_pool(name="pc_h", bufs=2) as pc_h,
                tc.tile_pool(name="pc_ps", bufs=1, space="PSUM") as pc_ps,
                tc.tile_pool(name="pc_tr", bufs=2, space="PSUM") as pc_tr,
            ):
                whh0, wih1, whh1 = [], [], []
                for k in range(NK):
                    for lst, src, nm in (
                        (whh0, whh0T_in, "whh0"),
                        (wih1, wih1T_in, "wih1"),
                        (whh1, whh1T_in, "whh1"),
                    ):
                        wt = pc_w.tile([128, G], FP16, tag=f"{nm}_{k}")
                        nc.sync.dma_start(wt[:], src[k * 128 : (k + 1) * 128, :])
                        lst.append(wt)
                b1rz = pc_w.tile([128, 2048], FP16, tag="b1rz")
                nc.sync.dma_start(b1rz[:], bias1rz_in[:])
                bi1n = pc_w.tile([128, HID], FP16, tag="bi1n")
                nc.sync.dma_start(bi1n[:], bih1n_in[:])
                bh0n = pc_w.tile([128, HID], FP16, tag="bh0n")
                nc.sync.dma_start(bh0n[:], bhh0n_in[:])
                bh1n = pc_w.tile([128, HID], FP16, tag="bh1n")
                nc.sync.dma_start(bh1n[:], bhh1n_in[:])

                # initial state
                hT = [[], []]
                for layer in range(2):
                    for k in range(NK):
                        t0 = pc_h.tile([128, B], FP16, tag=f"h{layer}T{k}")
                        nc.sync.dma_start(
                            t0[:], hT_in[layer, k * 128 : (k + 1) * 128, :]
                        )
                        hT[layer].append(t0)
                hcur = []
                for layer in range(2):
                    t0 = pc_h.tile([128, HID], F32, tag=f"h{layer}n")
                    nc.sync.dma_start(t0[:], h_in[layer])
                    hcur.append(t0)

                def gates_for_chunks(
                    layer, chunks, pa_ps_list, pb_ps_list, bias_rz, bi_n, bh_n,
                    gi_pre, rp, zp, hnew
                ):
                    """Gate math for given chunk indices. pa/pb lists are
                    indexed by position within `chunks` (pa may be None when
                    gi is precomputed in sbuf gi_pre)."""
                    for pos, nch in enumerate(chunks):
                        gate, piece = nch // 2, nch % 2
                        sl = slice(nch * 512, (nch + 1) * 512)
                        hsl = slice(piece * 512, (piece + 1) * 512)
                        pb = pb_ps_list[pos]
                        if gate < 2:
                            pre = pc_tmp.tile([128, 512], FP16, tag="pre")
                            if gi_pre is not None:
                                nc.vector.tensor_tensor(
                                    pre[:], pb[:], gi_pre[:, sl], op=ADD
                                )
                            else:
                                pa = pa_ps_list[pos]
                                t1 = pc_tmp.tile([128, 512], FP16, tag="t1")
                                nc.vector.tensor_tensor(
                                    t1[:], pa[:], bias_rz[:, sl], op=ADD
                                )
                                nc.vector.tensor_tensor(pre[:], t1[:], pb[:], op=ADD)
                            gout = pc_g.tile([128, 512], FP16, tag=f"g{gate}_{piece}")
                            nc.scalar.activation(gout[:], pre[:], AF.Sigmoid)
                            (rp if gate == 0 else zp)[piece] = gout
                        else:
                            # n gate: tanh(i_n + b_in + r * (h_n + b_hn))
                            hn = pc_tmp.tile([128, 512], FP16, tag="hn")
                            nc.vector.tensor_tensor(hn[:], pb[:], bh_n[:, hsl], op=ADD)
                            rt_ = pc_tmp.tile([128, 512], FP16, tag="rt")
                            nc.vector.tensor_tensor(
                                rt_[:], rp[piece][:], hn[:], op=MUL
                            )
                            pre = pc_tmp.tile([128, 512], FP16, tag="pre")
                            if gi_pre is not None:
                                nc.vector.tensor_tensor(
                                    pre[:], rt_[:], gi_pre[:, sl], op=ADD
                                )
                            else:
                                pa = pa_ps_list[pos]
                                in_ = pc_tmp.tile([128, 512], FP16, tag="t1")
                                nc.vector.tensor_tensor(
                                    in_[:], pa[:], bi_n[:, hsl], op=ADD
                                )
                                nc.vector.tensor_tensor(pre[:], rt_[:], in_[:], op=ADD)
                            nt = pc_tmp.tile([128, 512], FP16, tag="nt")
                            nc.scalar.activation(nt[:], pre[:], AF.Tanh)
                            # h_new = n + z * (h_old - n)
                            d = pc_tmp.tile([128, 512], FP16, tag="d")
                            nc.vector.tensor_tensor(
                                d[:], hcur[layer][:, hsl], nt[:], op=SUB
                            )
                            e = pc_tmp.tile([128, 512], FP16, tag="e")
                            nc.vector.tensor_tensor(e[:], zp[piece][:], d[:], op=MUL)
                            nc.vector.tensor_tensor(hnew[:, hsl], nt[:], e[:], op=ADD)

                def mm_group(ps_tags, hT_list, w_list, chunks):
                    """k-outer accumulation: for each k, one stationary load
                    feeds len(chunks) matmuls."""
                    pss = [
                        pc_ps.tile([128, 512], F32, tag=tg, name=f"mm_{tg}")
                        for tg in ps_tags
                    ]
                    for k in range(NK):
                        for pos, nch in enumerate(chunks):
                            nc.tensor.matmul(
                                pss[pos][:],
                                hT_list[k][:],
                                w_list[k][:, nch * 512 : (nch + 1) * 512],
                                start=(k == 0),
                                stop=(k == NK - 1),
                            )
                    return pss

                for t in range(t_steps):
                    gi0_sb = pc_pf.tile([128, G], FP16, tag="gi0")
                    j, row = (t % 8) // 2, (t // 8) * 256 + (t % 8 % 2) * 128
                    nc.sync.dma_start(gi0_sb[:], gi0_ch[j][row : row + 128, :])

                    # ---- layer 0 (two halves of 3 chunks) ----
                    h0n = pc_h.tile([128, HID], F32, tag="h0n")
                    rp0, zp0 = [None, None], [None, None]
                    for half in range(2):
                        chunks = [3 * half, 3 * half + 1, 3 * half + 2]
                        tg = "ca" if half == 0 else "cb"
                        pbs = mm_group(
                            [f"{tg}{i}" for i in range(3)], hT[0], whh0, chunks
                        )
                        gates_for_chunks(
                            0, chunks, None, pbs, None, None, bh0n, gi0_sb,
                            rp0, zp0, h0n
                        )

                    for k in range(NK):
                        ptr = pc_tr.tile([128, 128], F32, tag="tr")
                        nc.tensor.transpose(
                            ptr[:], h0n[:, k * 128 : (k + 1) * 128], ident[:]
                        )
                        nt_ = pc_h.tile([128, B], FP16, tag=f"h0T{k}")
                        nc.vector.tensor_copy(nt_[:], ptr[:])
                        hT[0][k] = nt_
                    hcur[0] = h0n

                    # ---- layer 1 (two halves; gi1 and gh1 separate psums) --
                    h1n = pc_h.tile([128, HID], F32, tag="h1n")
                    rp1, zp1 = [None, None], [None, None]
                    for half in range(2):
                        chunks = [3 * half, 3 * half + 1, 3 * half + 2]
                        pas = mm_group(
                            [f"ca{i}" for i in range(3)], hT[0], wih1, chunks
                        )
                        pbs = mm_group(
                            [f"cb{i}" for i in range(3)], hT[1], whh1, chunks
                        )
                        gates_for_chunks(
                            1, chunks, pas, pbs, b1rz, bi1n, bh1n, None,
                            rp1, zp1, h1n
                        )

                    # persist o_t = h1(t) in natural fp32 layout for attention
                    nc.sync.dma_start(o_nat[t * B : (t + 1) * B, :], h1n[:])
                    if t < t_steps - 1:
                        for k in range(NK):
                            ptr = pc_tr.tile([128, 128], F32, tag="tr")
                            nc.tensor.transpose(
                                ptr[:], h1n[:, k * 128 : (k + 1) * 128], ident[:]
                            )
                            nt_ = pc_h.tile([128, B], FP16, tag=f"h1T{k}")
                            nc.vector.tensor_copy(nt_[:], ptr[:])
                            hT[1][k] = nt_
                    hcur[1] = h1n

                # final h out (identical on every core)
                nc.sync.dma_start(hfin_dram[0], hcur[0][:])
                nc.sync.dma_start(hfin_dram[1], hcur[1][:])

            # ================= PHASE D: attention (my 16 batch) ========
            with (
                tc.tile_pool(name="pd_big", bufs=1) as pd_big,
                tc.tile_pool(name="pd_sb", bufs=2) as pd_sb,
                tc.tile_pool(name="pd_ps", bufs=1, space="PSUM") as pd_ps,
            ):
                # gather my (t, b-shard) rows of o and transpose on device
                oidx_sb = pd_big.tile([128, 8], mybir.dt.int32, tag="oidx")
                nc.sync.dma_start(oidx_sb[:], oidx_in[:])
                oT_all = pd_big.tile([128, NK, T, BSH], F32R, tag="oT_all")
                for m in range(8):
                    orows = pd_sb.tile([128, HID], F32, tag="orows")
                    nc.gpsimd.indirect_dma_start(
                        out=orows[:],
                        out_offset=None,
                        in_=o_nat[:],
                        in_offset=bass.IndirectOffsetOnAxis(
                            ap=oidx_sb[:, m : m + 1], axis=0
                        ),
                    )
                    for k in range(NK):
                        ptr = pd_ps.tile([128, 128], F32, tag="trq", bufs=2)
                        nc.tensor.transpose(
                            ptr[:], orows[:, k * 128 : (k + 1) * 128], ident[:]
                        )
                        nc.vector.tensor_copy(
                            oT_all[:, k, m * 8 : (m + 1) * 8, :], ptr[:]
                        )
                q1T_all = pd_big.tile([128, NK, T, BSH], F32, tag="q1T_all")
                ctxT_all = pd_big.tile([128, NK, T, BSH], F32R, tag="ctxT_all")

                # ---- q1 = o @ L1.T  (fp32r, k-outer) ----
                with tc.tile_pool(name="pd_l1", bufs=1) as pd_l1:
                    l1t = []
                    for k in range(NK):
                        wt = pd_l1.tile([128, HID], F32R, tag=f"l1t{k}")
                        nc.sync.dma_start(wt[:], L1T_in[k * 128 : (k + 1) * 128, :])
                        l1t.append(wt)
                    for m in range(8):  # row tiles: 8t x 16b
                        q1m = pd_sb.tile([128, HID], F32, tag="q1m")
                        ps2 = [
                            pd_ps.tile([128, 512], F32, tag=f"q{i}", name=f"ps2_{i}")
                            for i in range(2)
                        ]
                        for k in range(NK):
                            for i in range(2):
                                nc.tensor.matmul(
                                    ps2[i][:],
                                    oT_all[:, k, m * 8 : (m + 1) * 8, :],
                                    l1t[k][:, i * 512 : (i + 1) * 512],
                                    start=(k == 0),
                                    stop=(k == NK - 1),
                                )
                        for i in range(2):
                            nc.vector.tensor_copy(
                                q1m[:, i * 512 : (i + 1) * 512], ps2[i][:]
                            )
                        for k in range(NK):
                            ptr = pd_ps.tile([128, 128], F32, tag="trq", bufs=2)
                            nc.tensor.transpose(
                                ptr[:], q1m[:, k * 128 : (k + 1) * 128], ident[:]
                            )
                            nc.vector.tensor_copy(
                                q1T_all[:, k, m * 8 : (m + 1) * 8, :], ptr[:]
                            )

                # ---- per-b: scores, softmax, context (fp32) ----
                for b in range(BSH):
                    hct = []
                    for k in range(NK):
                        t_ = pd_sb.tile([128, S], F32, tag=f"hct{k}")
                        nc.sync.dma_start(
                            t_[:], HcT_in[b, k * 128 : (k + 1) * 128, :]
                        )
                        hct.append(t_)
                    hcb = pd_sb.tile([S, HID], F32, tag="hcb")
                    nc.sync.dma_start(hcb[:], Hc_in[b])

                    ps_s = pd_ps.tile([T, S], F32, tag="sc", bufs=2)
                    for k in range(NK):
                        nc.tensor.matmul(
                            ps_s[:],
                            q1T_all[:, k, :, b],
                            hct[k][:],
                            start=(k == 0),
                            stop=(k == NK - 1),
                        )
                    negmax = pd_sb.tile([T, 1], F32, tag="negmax")
                    nc.vector.tensor_reduce(
                        negmax[:], ps_s[:], axis=mybir.AxisListType.X,
                        op=AluOpType.max, negate=True,
                    )
                    ab = pd_sb.tile([T, S], F32, tag="ab")
                    ssum = pd_sb.tile([T, 1], F32, tag="ssum")
                    nc.scalar.activation(
                        ab[:], ps_s[:], AF.Exp, bias=negmax[:], accum_out=ssum[:]
                    )
                    rec = pd_sb.tile([T, 1], F32, tag="rec")
                    nc.vector.reciprocal(rec[:], ssum[:])
                    nc.vector.tensor_scalar_mul(ab[:], ab[:], rec[:])
                    ptr = pd_ps.tile([S, T], F32, tag="tra", bufs=1)
                    nc.tensor.transpose(ptr[:], ab[:], ident[:T, :T])
                    abT = pd_sb.tile([S, T], F32, tag="abT")
                    nc.vector.tensor_copy(abT[:], ptr[:])
                    for jj in range(NK):
                        pc_ = pd_ps.tile([128, T], F32, tag="ctx", bufs=1)
                        nc.tensor.matmul(
                            pc_[:],
                            hcb[:, jj * 128 : (jj + 1) * 128],
                            abT[:],
                            start=True,
                            stop=True,
                        )
                        nc.vector.tensor_copy(ctxT_all[:, jj, :, b], pc_[:])

                # ---- out = tanh([ctx, o] @ L2.T)  (fp32r, k-outer) ----
                with tc.tile_pool(name="pd_l2", bufs=1) as pd_l2:
                    l2t = []
                    for k2 in range(2 * NK):
                        wt = pd_l2.tile([128, HID], F32R, tag=f"l2t{k2}")
                        nc.sync.dma_start(
                            wt[:], L2T_in[k2 * 128 : (k2 + 1) * 128, :]
                        )
                        l2t.append(wt)
                    for m in range(8):
                        outm = pd_sb.tile([128, HID], F32, tag="outm")
                        ps2 = [
                            pd_ps.tile([128, 512], F32, tag=f"q{i}", name=f"ps2_{i}")
                            for i in range(2)
                        ]
                        for k2 in range(2 * NK):
                            lhs = (
                                ctxT_all[:, k2, m * 8 : (m + 1) * 8, :]
                                if k2 < NK
                                else oT_all[:, k2 - NK, m * 8 : (m + 1) * 8, :]
                            )
                            for i in range(2):
                                nc.tensor.matmul(
                                    ps2[i][:],
                                    lhs,
                                    l2t[k2][:, i * 512 : (i + 1) * 512],
                                    start=(k2 == 0),
                                    stop=(k2 == 2 * NK - 1),
                                )
                        for i in range(2):
                            nc.scalar.activation(
                                outm[:, i * 512 : (i + 1) * 512], ps2[i][:], AF.Tanh
                            )
                        nc.sync.dma_start(
                            out_dram[m * 8 : (m + 1) * 8, :, :].rearrange(
                                "t b h -> (t b) h"
                            ),
                            outm[:],
                        )

    nc.finalize()
    return nc


def _tf32(x):
    xi = x.astype(np.float32).view(np.uint32)
    return (xi & np.uint32(0xFFFFE000)).view(np.float32)


def _prep_inputs(input, h, H, emb, w_ih, w_hh, b_ih, b_hh, L1, L2):
    bf = np.float16
    input = np.asarray(input)
    h = np.asarray(h, dtype=np.float32)
    H = np.asarray(H, dtype=np.float32)
    emb = np.asarray(emb, dtype=np.float32)
    w_ih = np.asarray(w_ih, dtype=np.float32)
    w_hh = np.asarray(w_hh, dtype=np.float32)
    b_ih = np.asarray(b_ih, dtype=np.float32)
    b_hh = np.asarray(b_hh, dtype=np.float32)
    L1 = np.asarray(L1, dtype=np.float32)
    L2 = np.asarray(L2, dtype=np.float32)

    embed = emb[input]  # (T, B, EMB) host gather (pure indexing)

    rep = lambda v: np.ascontiguousarray(
        np.broadcast_to(v[None, :], (128, v.shape[0]))
    ).astype(bf)
    # layer0: b_hh r,z parts fold into gi0 bias; b_ih n part in gi0, b_hh n per-step
    bias0 = np.concatenate(
        [
            b_ih[0][:HID] + b_hh[0][:HID],
            b_ih[0][HID : 2 * HID] + b_hh[0][HID : 2 * HID],
            b_ih[0][2 * HID :],
        ]
    )
    bias1rz = b_ih[1][: 2 * HID] + b_hh[1][: 2 * HID]

    shared = {
        "wih0T": np.ascontiguousarray(w_ih[0].T).astype(bf),
        "whh0T": np.ascontiguousarray(w_hh[0].T).astype(bf),
        "wih1T": np.ascontiguousarray(w_ih[1].T).astype(bf),
        "whh1T": np.ascontiguousarray(w_hh[1].T).astype(bf),
        "bias0": rep(bias0),
        "bias1rz": rep(bias1rz),
        "bih1n": rep(b_ih[1][2 * HID :]),
        "bhh0n": rep(b_hh[0][2 * HID :]),
        "bhh1n": rep(b_hh[1][2 * HID :]),
        "hT": np.ascontiguousarray(np.transpose(h, (0, 2, 1))).astype(bf),
        "h0": h,
        "L1T": _tf32(np.ascontiguousarray(L1.T)),
        "L2T": _tf32(np.ascontiguousarray(L2.T)),
    }
    in_maps = []
    for c in range(NCORES):
        tsl = slice(c * TSH, (c + 1) * TSH)
        bsl = slice(c * BSH, (c + 1) * BSH)
        m = dict(shared)
        m["embT"] = np.ascontiguousarray(
            embed[tsl].reshape(TSH * B, EMB).T
        ).astype(bf)
        Hb = H[:, bsl, :]  # (S, BSH, HID)
        m["HcT"] = np.ascontiguousarray(np.transpose(Hb, (1, 2, 0)))
        m["Hc"] = np.ascontiguousarray(np.transpose(Hb, (1, 0, 2)))
        # row indices into o_nat [T*B, HID]: tile m row (tl*16+bl) ->
        # t = m*8+tl, b = c*16+bl
        tl = np.arange(8)
        bl = np.arange(BSH)
        oidx = np.empty((8, 128), dtype=np.int32)
        for mi in range(8):
            oidx[mi] = (
                ((mi * 8 + tl)[:, None] * B) + (c * BSH + bl)[None, :]
            ).reshape(-1)
        m["oidx"] = np.ascontiguousarray(oidx.T)  # [128, 8]
        in_maps.append(m)
    return in_maps


def kernel(input, h, H, emb, w_ih, w_hh, b_ih, b_hh, L1, L2):
    if "nc" not in _BUILD_CACHE:
        _BUILD_CACHE["nc"] = build()
    nc = _BUILD_CACHE["nc"]
    in_maps = _prep_inputs(input, h, H, emb, w_ih, w_hh, b_ih, b_hh, L1, L2)
    res = run_bass_kernel_spmd(
        nc, in_maps, core_ids=list(range(NCORES)), trace=False
    )
    output = np.empty((T, B, HID), dtype=np.float32)
    for c in range(NCORES):
        output[:, c * BSH : (c + 1) * BSH, :] = res.results[c]["out"]
    h_final = res.results[0]["hfin"]
    return output, h_final


# revision 24
# speedup vs baseline: 1.0240x; 1.0240x over previous
"""Trainium2 Bass kernel for nn_Decoder_without_dropout (2-layer GRU + attention).

Strategy (8 NeuronCores):
- The GRU recurrence (T=64 steps) is inherently sequential and its per-step
  matmuls stream the full weight matrices through the PE regardless of batch,
  so it is REPLICATED on all 8 cores (per-step cross-core collectives measured
  ~13us each - more than the redundant compute they would save).
- Everything parallel is sharded: layer-0 input projections gi0 by time
  (8 steps/core, gathered via 4 chunked AllGathers that overlap compute),
  attention + output projection by batch (16 elems/core).
- The attention/output block never feeds the recurrence, so it is deferred
  and batched after the 64 steps.
- dtypes: recurrence matmuls fp16 (all values are small, so fp16's 11-bit
  mantissa beats bf16 8x in noise; sim-validated out err ~8e-3), state
  carried fp32, attention scores/softmax/context in plain fp32 (softmax
  amplifies score noise), q1/output projection in fp32r (HW-measured
  1.4e-4 rel err).
"""

import sys

import numpy as np

sys.path.insert(0, "/opt/trn_rl_repo")

import concourse.bass as bass  # noqa: E402
import concourse.mybir as mybir  # noqa: E402
import concourse.tile as tile  # noqa: E402
from concourse import bacc  # noqa: E402
from concourse.alu_op_type import AluOpType  # noqa: E402
from concourse.bass_utils import run_bass_kernel_spmd  # noqa: E402
from concourse.masks import make_identity  # noqa: E402

T, B, HID, EMB = 64, 128, 1024, 1024
G = 3 * HID  # 3072 gate width
S = 128  # attention source length
NCORES = 8
TSH = T // NCORES  # 8 timesteps per core (gi0 shard)
BSH = B // NCORES  # 16 batch elems per core (attention shard)
NK = HID // 128  # 8 contraction chunks
NAG = 8  # gi0 AllGather chunks (one per local row-tile)

F32 = mybir.dt.float32
F32R = mybir.dt.float32r
FP16 = mybir.dt.float16
AF = mybir.ActivationFunctionType
ADD = AluOpType.add
SUB = AluOpType.subtract
MUL = AluOpType.mult

_BUILD_CACHE = {}


def build(t_steps=T):
    nc = bacc.Bacc(
        "TRN2", target_bir_lowering=False, debug=False, num_devices=NCORES
    )

    # ---- inputs (per-core) ----
    embT_in = nc.dram_tensor("embT", [EMB, TSH * B], FP16, kind="ExternalInput")
    wih0T_in = nc.dram_tensor("wih0T", [HID, G], FP16, kind="ExternalInput")
    whh0T_in = nc.dram_tensor("whh0T", [HID, G], FP16, kind="ExternalInput")
    wih1T_in = nc.dram_tensor("wih1T", [HID, G], FP16, kind="ExternalInput")
    whh1T_in = nc.dram_tensor("whh1T", [HID, G], FP16, kind="ExternalInput")
    bias0_in = nc.dram_tensor("bias0", [128, G], FP16, kind="ExternalInput")
    bias1rz_in = nc.dram_tensor("bias1rz", [128, 2048], FP16, kind="ExternalInput")
    bih1n_in = nc.dram_tensor("bih1n", [128, HID], FP16, kind="ExternalInput")
    bhh0n_in = nc.dram_tensor("bhh0n", [128, HID], FP16, kind="ExternalInput")
    bhh1n_in = nc.dram_tensor("bhh1n", [128, HID], FP16, kind="ExternalInput")
    hT_in = nc.dram_tensor("hT", [2, HID, B], FP16, kind="ExternalInput")
    h_in = nc.dram_tensor("h0", [2, B, HID], F32, kind="ExternalInput")
    L1T_in = nc.dram_tensor("L1T", [HID, HID], F32R, kind="ExternalInput")
    L2T_in = nc.dram_tensor("L2T", [2 * HID, HID], F32R, kind="ExternalInput")
    # HcT[b][h][s] = H[s, bsl[b], h]   (scores rhs)
    HcT_in = nc.dram_tensor("HcT", [BSH, HID, S], F32, kind="ExternalInput")
    # Hc[b][s][h] = H[s, bsl[b], h]    (context lhsT)
    Hc_in = nc.dram_tensor("Hc", [BSH, S, HID], F32, kind="ExternalInput")
    # per-core row indices into o_nat [T*B, HID]: column m holds the 128 rows
    # (8 timesteps x 16 local batch) of phase-D row-tile m
    oidx_in = nc.dram_tensor("oidx", [128, 8], mybir.dt.int32, kind="ExternalInput")

    # ---- outputs ----
    out_dram = nc.dram_tensor("out", [T, BSH, HID], F32, kind="ExternalOutput")
    hfin_dram = nc.dram_tensor("hfin", [2, B, HID], F32, kind="ExternalOutput")

    with tile.TileContext(nc, num_cores=NCORES) as tc:
        with (
            tc.tile_pool(name="dram", bufs=1, space="DRAM") as dram,
            tc.tile_pool(name="consts", bufs=1) as consts,
        ):
            gi0_self = dram.tile([TSH * B, G], FP16)
            # stride-8 t-sharding: core c computes t in {c, c+8, ...}; its
            # row-tile j holds t = 8j + c, so AG chunk j (row-tile j of every
            # core, rank-major) holds global t in [8j, 8j+8): step t reads
            # chunk t//8 at row (t%8)*128. AG j fires right after row-tile j,
            # so the recurrence pipeline starts after the first small AG.
            gi0_ch = [
                dram.tile(
                    [NCORES * 128, G], FP16, addr_space="Shared", name=f"gi0ch{j}"
                )
                for j in range(NAG)
            ]
            o_nat = dram.tile([T * B, HID], F32)  # o_t = h1(t), natural layout

            ident = consts.tile([128, 128], F32)
            make_identity(nc, ident)

            # Preload the recurrence weights needed first (whh0 for L0,
            # wih1 for gi1) + per-step biases, so step 0 isn't gated on
            # 19MB of weight DMA behind phase A's SBUF release. whh1 loads
            # into the space phase A frees.
            pc_w = tc.alloc_tile_pool(name="pc_w", bufs=1)
            whh0, wih1 = [], []
            for k in range(NK):
                wt0 = pc_w.tile([128, G], FP16, tag=f"whh0_{k}", name=f"whh0_{k}")
                nc.sync.dma_start(wt0[:], whh0T_in[k * 128 : (k + 1) * 128, :])
                whh0.append(wt0)
                wt1 = pc_w.tile([128, G], FP16, tag=f"wih1_{k}", name=f"wih1_{k}")
                nc.sync.dma_start(wt1[:], wih1T_in[k * 128 : (k + 1) * 128, :])
                wih1.append(wt1)
            b1rz = pc_w.tile([128, 2048], FP16, tag="b1rz")
            nc.sync.dma_start(b1rz[:], bias1rz_in[:])
            bi1n = pc_w.tile([128, HID], FP16, tag="bi1n")
            nc.sync.dma_start(bi1n[:], bih1n_in[:])
            bh0n = pc_w.tile([128, HID], FP16, tag="bh0n")
            nc.sync.dma_start(bh0n[:], bhh0n_in[:])
            bh1n = pc_w.tile([128, HID], FP16, tag="bh1n")
            nc.sync.dma_start(bh1n[:], bhh1n_in[:])

            # ========= PHASE A: gi0 for my 8 timesteps + chunked AG ====
            with (
                tc.tile_pool(name="pa_w", bufs=1) as pa_w,
                tc.tile_pool(name="pa_sb", bufs=2) as pa_sb,
                tc.tile_pool(name="pa_ps", bufs=1, space="PSUM") as pa_ps,
            ):
                wih0 = []
                for k in range(NK):
                    wt = pa_w.tile([128, G], FP16, tag=f"wih0_{k}")
                    nc.sync.dma_start(wt[:], wih0T_in[k * 128 : (k + 1) * 128, :])
                    wih0.append(wt)
                bias0 = pa_w.tile([128, G], FP16, tag="bias0")
                nc.sync.dma_start(bias0[:], bias0_in[:])

                for rt in range(TSH):  # row tiles of 128 (one t, all b)
                    ets = []
                    for k in range(NK):
                        et = pa_sb.tile([128, 128], FP16, tag=f"et{k}")
                        nc.sync.dma_start(
                            et[:],
                            embT_in[k * 128 : (k + 1) * 128, rt * 128 : (rt + 1) * 128],
                        )
                        ets.append(et)
                    # k-outer: one stationary load serves all 6 output chunks
                    ps6 = [
                        pa_ps.tile([128, 512], F32, tag=f"pa{i}", name=f"ps6_{i}")
                        for i in range(6)
                    ]
                    for k in range(NK):
                        for i in range(6):
                            nc.tensor.matmul(
                                ps6[i][:],
                                ets[k][:],
                                wih0[k][:, i * 512 : (i + 1) * 512],
                                start=(k == 0),
                                stop=(k == NK - 1),
                            )
                    for i in range(6):
                        ob = pa_sb.tile([128, 512], FP16, tag="ob")
                        nc.vector.tensor_tensor(
                            ob[:], ps6[i][:], bias0[:, i * 512 : (i + 1) * 512], op=ADD
                        )
                        nc.sync.dma_start(
                            gi0_self[
                                rt * 128 : (rt + 1) * 128, i * 512 : (i + 1) * 512
                            ],
                            ob[:],
                        )
                    nc.gpsimd.collective_compute(
                        "AllGather",
                        AluOpType.bypass,
                        replica_groups=[list(range(NCORES))],
                        ins=[gi0_self[128 * rt : 128 * (rt + 1), :]],
                        outs=[gi0_ch[rt].opt()],
                    )

            # ================= PHASE C: recurrence =====================
            with (
                tc.tile_pool(name="pc_w2", bufs=1) as pc_w2,
                tc.tile_pool(name="pc_pf", bufs=1) as pc_pf,
                tc.tile_pool(name="pc_g", bufs=1) as pc_g,
                tc.tile_pool(name="pc_tmp", bufs=1) as pc_tmp,
                tc.tile_pool(name="pc_h", bufs=2) as pc_h,
                tc.tile_pool(name="pc_ps", bufs=1, space="PSUM") as pc_ps,
                tc.tile_pool(name="pc_tr", bufs=2, space="PSUM") as pc_tr,
            ):
                whh1 = []
                for k in range(NK):
                    wt = pc_w2.tile([128, G], FP16, tag=f"whh1_{k}", name=f"whh1_{k}")
                    nc.sync.dma_start(wt[:], whh1T_in[k * 128 : (k + 1) * 128, :])
                    whh1.append(wt)

                # initial state
                hT = [[], []]
                for layer in range(2):
                    for k in range(NK):
                        t0 = pc_h.tile([128, B], FP16, tag=f"h{layer}T{k}")
                        nc.sync.dma_start(
                            t0[:], hT_in[layer, k * 128 : (k + 1) * 128, :]
                        )
                        hT[layer].append(t0)
                hcur = []
                for layer in range(2):
                    t0 = pc_h.tile([128, HID], F32, tag=f"h{layer}n")
                    nc.sync.dma_start(t0[:], h_in[layer])
                    hcur.append(t0)

                def gates_for_chunks(
                    layer, chunks, pa_ps_list, pb_ps_list, bias_rz, bi_n, bh_n,
                    gi_pre, rp, zp, hnew
                ):
                    """Gate math for given chunk indices. pa/pb lists are
                    indexed by position within `chunks` (pa may be None when
                    gi is precomputed in sbuf gi_pre)."""
                    for pos, nch in enumerate(chunks):
                        gate, piece = nch // 2, nch % 2
                        sl = slice(nch * 512, (nch + 1) * 512)
                        hsl = slice(piece * 512, (piece + 1) * 512)
                        pb = pb_ps_list[pos]
                        if gate < 2:
                            pre = pc_tmp.tile([128, 512], FP16, tag="pre")
                            if gi_pre is not None:
                                nc.vector.tensor_tensor(
                                    pre[:], pb[:], gi_pre[:, sl], op=ADD
                                )
                            else:
                                pa = pa_ps_list[pos]
                                t1 = pc_tmp.tile([128, 512], FP16, tag="t1")
                                nc.vector.tensor_tensor(
                                    t1[:], pa[:], bias_rz[:, sl], op=ADD
                                )
                                nc.vector.tensor_tensor(pre[:], t1[:], pb[:], op=ADD)
                            gout = pc_g.tile([128, 512], FP16, tag=f"g{gate}_{piece}")
                            nc.scalar.activation(gout[:], pre[:], AF.Sigmoid)
                            (rp if gate == 0 else zp)[piece] = gout
                        else:
                            # n gate: tanh(i_n + b_in + r * (h_n + b_hn))
                            hn = pc_tmp.tile([128, 512], FP16, tag="hn")
                            nc.vector.tensor_tensor(hn[:], pb[:], bh_n[:, hsl], op=ADD)
                            rt_ = pc_tmp.tile([128, 512], FP16, tag="rt")
                            nc.vector.tensor_tensor(
                                rt_[:], rp[piece][:], hn[:], op=MUL
                            )
                            pre = pc_tmp.tile([128, 512], FP16, tag="pre")
                            if gi_pre is not None:
                                nc.vector.tensor_tensor(
                                    pre[:], rt_[:], gi_pre[:, sl], op=ADD
                                )
                            else:
                                pa = pa_ps_list[pos]
                                in_ = pc_tmp.tile([128, 512], FP16, tag="t1")
                                nc.vector.tensor_tensor(
                                    in_[:], pa[:], bi_n[:, hsl], op=ADD
                                )
                                nc.vector.tensor_tensor(pre[:], rt_[:], in_[:], op=ADD)
                            nt = pc_tmp.tile([128, 512], FP16, tag="nt")
                            nc.scalar.activation(nt[:], pre[:], AF.Tanh)
                            # h_new = n + z * (h_old - n)
                            d = pc_tmp.tile([128, 512], FP16, tag="d")
                            nc.vector.tensor_tensor(
                                d[:], hcur[layer][:, hsl], nt[:], op=SUB
                            )
                            e = pc_tmp.tile([128, 512], FP16, tag="e")
                            nc.vector.tensor_tensor(e[:], zp[piece][:], d[:], op=MUL)
                            nc.vector.tensor_tensor(hnew[:, hsl], nt[:], e[:], op=ADD)

                def mm_group(ps_tags, hT_list, w_list, chunks):
                    """k-outer accumulation: for each k, one stationary load
                    feeds len(chunks) matmuls."""
                    pss = [
                        pc_ps.tile([128, 512], F32, tag=tg, name=f"mm_{tg}")
                        for tg in ps_tags
                    ]
                    for k in range(NK):
                        for pos, nch in enumerate(chunks):
                            nc.tensor.matmul(
                                pss[pos][:],
                                hT_list[k][:],
                                w_list[k][:, nch * 512 : (nch + 1) * 512],
                                start=(k == 0),
                                stop=(k == NK - 1),
                            )
                    return pss

                for t in range(t_steps):
                    gi0_sb = pc_pf.tile([128, G], FP16, tag="gi0")
                    j, row = t // 8, (t % 8) * 128
                    nc.sync.dma_start(gi0_sb[:], gi0_ch[j][row : row + 128, :])

                    # ---- layer 0 (two halves of 3 chunks) ----
                    h0n = pc_h.tile([128, HID], F32, tag="h0n")
                    rp0, zp0 = [None, None], [None, None]
                    for half in range(2):
                        chunks = [3 * half, 3 * half + 1, 3 * half + 2]
                        tg = "ca" if half == 0 else "cb"
                        pbs = mm_group(
                            [f"{tg}{i}" for i in range(3)], hT[0], whh0, chunks
                        )
                        gates_for_chunks(
                            0, chunks, None, pbs, None, None, bh0n, gi0_sb,
                            rp0, zp0, h0n
                        )

                    for k in range(NK):
                        ptr = pc_tr.tile([128, 128], F32, tag="tr")
                        nc.tensor.transpose(
                            ptr[:], h0n[:, k * 128 : (k + 1) * 128], ident[:]
                        )
                        nt_ = pc_h.tile([128, B], FP16, tag=f"h0T{k}")
                        nc.vector.tensor_copy(nt_[:], ptr[:])
                        hT[0][k] = nt_
                    hcur[0] = h0n

                    # ---- layer 1 (two halves; gi1 and gh1 separate psums) --
                    h1n = pc_h.tile([128, HID], F32, tag="h1n")
                    rp1, zp1 = [None, None], [None, None]
                    for half in range(2):
                        chunks = [3 * half, 3 * half + 1, 3 * half + 2]
                        pas = mm_group(
                            [f"ca{i}" for i in range(3)], hT[0], wih1, chunks
                        )
                        pbs = mm_group(
                            [f"cb{i}" for i in range(3)], hT[1], whh1, chunks
                        )
                        gates_for_chunks(
                            1, chunks, pas, pbs, b1rz, bi1n, bh1n, None,
                            rp1, zp1, h1n
                        )

                    # persist o_t = h1(t) in natural fp32 layout for attention
                    nc.sync.dma_start(o_nat[t * B : (t + 1) * B, :], h1n[:])
                    if t < t_steps - 1:
                        for k in range(NK):
                            ptr = pc_tr.tile([128, 128], F32, tag="tr")
                            nc.tensor.transpose(
                                ptr[:], h1n[:, k * 128 : (k + 1) * 128], ident[:]
                            )
                            nt_ = pc_h.tile([128, B], FP16, tag=f"h1T{k}")
                            nc.vector.tensor_copy(nt_[:], ptr[:])
                            hT[1][k] = nt_
                    hcur[1] = h1n

                # final h out (identical on every core)
                nc.sync.dma_start(hfin_dram[0], hcur[0][:])
                nc.sync.dma_start(hfin_dram[1], hcur[1][:])

            pc_w.release()

            # ================= PHASE D: attention (my 16 batch) ========
            with (
                tc.tile_pool(name="pd_big", bufs=1) as pd_big,
                tc.tile_pool(name="pd_sb", bufs=2) as pd_sb,
                tc.tile_pool(name="pd_ps", bufs=1, space="PSUM") as pd_ps,
            ):
                # gather my (t, b-shard) rows of o and transpose on device
                oidx_sb = pd_big.tile([128, 8], mybir.dt.int32, tag="oidx")
                nc.sync.dma_start(oidx_sb[:], oidx_in[:])
                oT_all = pd_big.tile([128, NK, T, BSH], F32R, tag="oT_all")
                for m in range(8):
                    orows = pd_sb.tile([128, HID], F32, tag="orows")
                    nc.gpsimd.indirect_dma_start(
                        out=orows[:],
                        out_offset=None,
                        in_=o_nat[:],
                        in_offset=bass.IndirectOffsetOnAxis(
                            ap=oidx_sb[:, m : m + 1], axis=0
                        ),
                    )
                    for k in range(NK):
                        ptr = pd_ps.tile([128, 128], F32, tag="trq", bufs=2)
                        nc.tensor.transpose(
                            ptr[:], orows[:, k * 128 : (k + 1) * 128], ident[:]
                        )
                        nc.vector.tensor_copy(
                            oT_all[:, k, m * 8 : (m + 1) * 8, :], ptr[:]
                        )
                q1T_all = pd_big.tile([128, NK, T, BSH], F32, tag="q1T_all")
                ctxT_all = pd_big.tile([128, NK, T, BSH], F32R, tag="ctxT_all")

                # ---- q1 = o @ L1.T  (fp32r, k-outer) ----
                with tc.tile_pool(name="pd_l1", bufs=1) as pd_l1:
                    l1t = []
                    for k in range(NK):
                        wt = pd_l1.tile([128, HID], F32R, tag=f"l1t{k}")
                        nc.sync.dma_start(wt[:], L1T_in[k * 128 : (k + 1) * 128, :])
                        l1t.append(wt)
                    for m in range(8):  # row tiles: 8t x 16b
                        q1m = pd_sb.tile([128, HID], F32, tag="q1m")
                        ps2 = [
                            pd_ps.tile([128, 512], F32, tag=f"q{i}", name=f"ps2_{i}")
                            for i in range(2)
                        ]
                        for k in range(NK):
                            for i in range(2):
                                nc.tensor.matmul(
                                    ps2[i][:],
                                    oT_all[:, k, m * 8 : (m + 1) * 8, :],
                                    l1t[k][:, i * 512 : (i + 1) * 512],
                                    start=(k == 0),
                                    stop=(k == NK - 1),
                                )
                        for i in range(2):
                            nc.vector.tensor_copy(
                                q1m[:, i * 512 : (i + 1) * 512], ps2[i][:]
                            )
                        for k in range(NK):
                            ptr = pd_ps.tile([128, 128], F32, tag="trq", bufs=2)
                            nc.tensor.transpose(
                                ptr[:], q1m[:, k * 128 : (k + 1) * 128], ident[:]
                            )
                            nc.vector.tensor_copy(
                                q1T_all[:, k, m * 8 : (m + 1) * 8, :], ptr[:]
                            )

                # ---- per-b: scores, softmax, context (fp32) ----
                for b in range(BSH):
                    hct = []
                    for k in range(NK):
                        t_ = pd_sb.tile([128, S], F32, tag=f"hct{k}")
                        nc.sync.dma_start(
                            t_[:], HcT_in[b, k * 128 : (k + 1) * 128, :]
                        )
                        hct.append(t_)
                    hcb = pd_sb.tile([S, HID], F32, tag="hcb")
                    nc.sync.dma_start(hcb[:], Hc_in[b])

                    ps_s = pd_ps.tile([T, S], F32, tag="sc", bufs=2)
                    for k in range(NK):
                        nc.tensor.matmul(
                            ps_s[:],
                            q1T_all[:, k, :, b],
                            hct[k][:],
                            start=(k == 0),
                            stop=(k == NK - 1),
                        )
                    negmax = pd_sb.tile([T, 1], F32, tag="negmax")
                    nc.vector.tensor_reduce(
                        negmax[:], ps_s[:], axis=mybir.AxisListType.X,
                        op=AluOpType.max, negate=True,
                    )
                    ab = pd_sb.tile([T, S], F32, tag="ab")
                    ssum = pd_sb.tile([T, 1], F32, tag="ssum")
                    nc.scalar.activation(
                        ab[:], ps_s[:], AF.Exp, bias=negmax[:], accum_out=ssum[:]
                    )
                    rec = pd_sb.tile([T, 1], F32, tag="rec")
                    nc.vector.reciprocal(rec[:], ssum[:])
                    nc.vector.tensor_scalar_mul(ab[:], ab[:], rec[:])
                    ptr = pd_ps.tile([S, T], F32, tag="tra", bufs=1)
                    nc.tensor.transpose(ptr[:], ab[:], ident[:T, :T])
                    abT = pd_sb.tile([S, T], F32, tag="abT")
                    nc.vector.tensor_copy(abT[:], ptr[:])
                    for jj in range(NK):
                        pc_ = pd_ps.tile([128, T], F32, tag="ctx", bufs=1)
                        nc.tensor.matmul(
                            pc_[:],
                            hcb[:, jj * 128 : (jj + 1) * 128],
                            abT[:],
                            start=True,
                            stop=True,
                        )
                        nc.vector.tensor_copy(ctxT_all[:, jj, :, b], pc_[:])

                # ---- out = tanh([ctx, o] @ L2.T)  (fp32r, k-outer) ----
                with tc.tile_pool(name="pd_l2", bufs=1) as pd_l2:
                    l2t = []
                    for k2 in range(2 * NK):
                        wt = pd_l2.tile([128, HID], F32R, tag=f"l2t{k2}")
                        nc.sync.dma_start(
                            wt[:], L2T_in[k2 * 128 : (k2 + 1) * 128, :]
                        )
                        l2t.append(wt)
                    for m in range(8):
                        outm = pd_sb.tile([128, HID], F32, tag="outm")
                        ps2 = [
                            pd_ps.tile([128, 512], F32, tag=f"q{i}", name=f"ps2_{i}")
                            for i in range(2)
                        ]
                        for k2 in range(2 * NK):
                            lhs = (
                                ctxT_all[:, k2, m * 8 : (m + 1) * 8, :]
                                if k2 < NK
                                else oT_all[:, k2 - NK, m * 8 : (m + 1) * 8, :]
                            )
                            for i in range(2):
                                nc.tensor.matmul(
                                    ps2[i][:],
                                    lhs,
                                    l2t[k2][:, i * 512 : (i + 1) * 512],
                                    start=(k2 == 0),
                                    stop=(k2 == 2 * NK - 1),
                                )
                        for i in range(2):
                            nc.scalar.activation(
                                outm[:, i * 512 : (i + 1) * 512], ps2[i][:], AF.Tanh
                            )
                        nc.sync.dma_start(
                            out_dram[m * 8 : (m + 1) * 8, :, :].rearrange(
                                "t b h -> (t b) h"
                            ),
                            outm[:],
                        )

    nc.finalize()
    return nc


def _tf32(x):
    xi = x.astype(np.float32).view(np.uint32)
    return (xi & np.uint32(0xFFFFE000)).view(np.float32)


def _prep_inputs(input, h, H, emb, w_ih, w_hh, b_ih, b_hh, L1, L2):
    bf = np.float16
    input = np.asarray(input)
    h = np.asarray(h, dtype=np.float32)
    H = np.asarray(H, dtype=np.float32)
    emb = np.asarray(emb, dtype=np.float32)
    w_ih = np.asarray(w_ih, dtype=np.float32)
    w_hh = np.asarray(w_hh, dtype=np.float32)
    b_ih = np.asarray(b_ih, dtype=np.float32)
    b_hh = np.asarray(b_hh, dtype=np.float32)
    L1 = np.asarray(L1, dtype=np.float32)
    L2 = np.asarray(L2, dtype=np.float32)

    embed = emb[input]  # (T, B, EMB) host gather (pure indexing)

    rep = lambda v: np.ascontiguousarray(
        np.broadcast_to(v[None, :], (128, v.shape[0]))
    ).astype(bf)
    # layer0: b_hh r,z parts fold into gi0 bias; b_ih n part in gi0, b_hh n per-step
    bias0 = np.concatenate(
        [
            b_ih[0][:HID] + b_hh[0][:HID],
            b_ih[0][HID : 2 * HID] + b_hh[0][HID : 2 * HID],
            b_ih[0][2 * HID :],
        ]
    )
    bias1rz = b_ih[1][: 2 * HID] + b_hh[1][: 2 * HID]

    shared = {
        "wih0T": np.ascontiguousarray(w_ih[0].T).astype(bf),
        "whh0T": np.ascontiguousarray(w_hh[0].T).astype(bf),
        "wih1T": np.ascontiguousarray(w_ih[1].T).astype(bf),
        "whh1T": np.ascontiguousarray(w_hh[1].T).astype(bf),
        "bias0": rep(bias0),
        "bias1rz": rep(bias1rz),
        "bih1n": rep(b_ih[1][2 * HID :]),
        "bhh0n": rep(b_hh[0][2 * HID :]),
        "bhh1n": rep(b_hh[1][2 * HID :]),
        "hT": np.ascontiguousarray(np.transpose(h, (0, 2, 1))).astype(bf),
        "h0": h,
        "L1T": _tf32(np.ascontiguousarray(L1.T)),
        "L2T": _tf32(np.ascontiguousarray(L2.T)),
    }
    in_maps = []
    for c in range(NCORES):
        tidx = np.arange(TSH) * NCORES + c  # stride-8 t shard
        bsl = slice(c * BSH, (c + 1) * BSH)
        m = dict(shared)
        m["embT"] = np.ascontiguousarray(
            embed[tidx].reshape(TSH * B, EMB).T
        ).astype(bf)
        Hb = H[:, bsl, :]  # (S, BSH, HID)
        m["HcT"] = np.ascontiguousarray(np.transpose(Hb, (1, 2, 0)))
        m["Hc"] = np.ascontiguousarray(np.transpose(Hb, (1, 0, 2)))
        # row indices into o_nat [T*B, HID]: tile m row (tl*16+bl) ->
        # t = m*8+tl, b = c*16+bl
        tl = np.arange(8)
        bl = np.arange(BSH)
        oidx = np.empty((8, 128), dtype=np.int32)
        for mi in range(8):
            oidx[mi] = (
                ((mi * 8 + tl)[:, None] * B) + (c * BSH + bl)[None, :]
            ).reshape(-1)
        m["oidx"] = np.ascontiguousarray(oidx.T)  # [128, 8]
        in_maps.append(m)
    return in_maps


def kernel(input, h, H, emb, w_ih, w_hh, b_ih, b_hh, L1, L2):
    if "nc" not in _BUILD_CACHE:
        _BUILD_CACHE["nc"] = build()
    nc = _BUILD_CACHE["nc"]
    in_maps = _prep_inputs(input, h, H, emb, w_ih, w_hh, b_ih, b_hh, L1, L2)
    res = run_bass_kernel_spmd(
        nc, in_maps, core_ids=list(range(NCORES)), trace=False
    )
    output = np.empty((T, B, HID), dtype=np.float32)
    for c in range(NCORES):
        output[:, c * BSH : (c + 1) * BSH, :] = res.results[c]["out"]
    h_final = res.results[0]["hfin"]
    return output, h_final
